# revision 1
# baseline (speedup 1.0000x reference)
"""Trainium2 Bass kernel for nn_Attn_58669253263845 (sparse_attention).

Reference computation:
    hidden2 = concat(hidden[0], hidden[1])                 # [B, 2H]
    attn_input = concat(bcast(hidden2), encoder_outputs)   # [B, S, 3H]
    energy = attn_input @ W.T + b                          # [B, S, H]
    scores = energy @ v                                    # [B, S]
    out = softmax(scores, axis=S)

Everything before the softmax is linear, so
    scores[b,s] = attn_input[b,s,:] . (v @ W) + v.b
                = hidden2[b,:] . w_hid + enc[b,s,:] . w_enc + v.b
The hidden/bias terms are constant per batch row and cancel in the softmax
over S.  Hence:
    out = softmax_s(enc[b,s,:] . w_enc),  w_enc = v @ W[:, 2H:3H]

The weight fold (1024x1024 matvec) is done on host in fp64; the heavy part
(64*512 dot products of length 1024 + softmax) runs on 8 NeuronCores,
data-parallel over batch (8 batches per core).
"""

import sys
import types

import numpy as np
import concourse.bacc as bacc
import concourse.bass as bass
import concourse.mybir as mybir
import concourse.tile as tile
from concourse.bass_utils import run_bass_kernel_spmd

# run_bass_kernel_spmd(trace=True) (e.g. via BASS_TRACE=1 in the env)
# imports antenv.axon_hooks, which does not exist in this container. Register
# a stub returning "no hook" so tracing degrades gracefully instead of
# raising ModuleNotFoundError.
try:
    import antenv.axon_hooks  # noqa: F401
except ImportError:
    try:
        import antenv

        _stub = types.ModuleType("antenv.axon_hooks")
        _stub.get_axon_ntff_profile_hook = lambda: None  # type: ignore[attr-defined]
        sys.modules["antenv.axon_hooks"] = _stub
        antenv.axon_hooks = _stub
    except ImportError:
        pass

N_CORES = 8
B, S, H = 64, 512, 1024
P = 128            # SBUF partitions
BPC = B // N_CORES  # batches per core = 8
JT = S // P         # s-chunks per batch = 4

F32 = mybir.dt.float32

_compiled_nc = None
LAST_RESULTS = None  # BassKernelResults of the most recent run (for profiling)

# knobs (read at build time)
# One SBUF tile per chunk (~16MiB of the 24MiB SBUF): zero slot reuse means
# zero WAW waits -> no legalized EventSemaphore stalls on the DVE sequencer.
EBUF_BUFS = 38
TAIL_CHUNKS = 6  # how many trailing chunks get their dot split in half
LAST_SPLIT = 2  # pieces for the very last chunk (partial-tile width)
# Size of the final piece of the last chunk. 512 (symmetric halves) is
# optimal: sem-propagation (900ns) exceeds a half-chunk transfer (728ns),
# so shrinking the last piece below half only delays its predecessor.
LAST_PIECE = 512
SCORES_PAD = 8  # f32 per score-accumulator slot (32B dep-tracking granule)


def _build_nc(ebuf_bufs=None, dma_only=False, compute_only=False):
    """Per-core kernel: probs[BPC, S] = softmax_s(enc[BPC, S, H] @ w_enc).

    dma_only / compute_only build crippled variants for cost attribution.
    """
    # Bacc (not raw Bass): its compile() legalizes multi-wait instructions
    # into EventSemaphore waits (TRN2 allows only 1 sync wait per inst).
    nc = bacc.Bacc("TRN2", target_bir_lowering=False, debug=False)

    enc_d = nc.dram_tensor("enc_in", [BPC, S, H], F32, kind="ExternalInput")
    w_d = nc.dram_tensor("w_in", [1, H], F32, kind="ExternalInput")
    out_d = nc.dram_tensor("probs_out", [BPC, S], F32, kind="ExternalOutput")

    enc = enc_d.ap()

    with tile.TileContext(nc) as tc:
        with (
            tc.tile_pool(name="const", bufs=1) as constp,
            tc.tile_pool(name="ebuf", bufs=ebuf_bufs or EBUF_BUFS) as ebufp,
            tc.tile_pool(name="small", bufs=1) as smallp,
            tc.tile_pool(name="psum", bufs=1, space="PSUM") as psump,
        ):
            # The first enc chunk goes first in the DMA stream: every other
            # DMA's descriptor-gen then hides behind a running transfer.
            et0 = ebufp.tile([P, H], F32, name="et", tag="et")
            if compute_only:
                nc.sync.dma_start(et0[0:1, 0:1], enc[0, 0:1, 0:1])
            else:
                nc.sync.dma_start(et0[:], enc[0, 0:P, :])

            # w arrives as a single row (4KiB) and is broadcast to all 128
            # partitions on-device via a K=1 matmul with a ones row -- much
            # cheaper than DMAing a host-replicated 512KiB copy. It is then
            # copied once from PSUM to SBUF: PSUM dependency tracking is
            # bank-granular and serializes successive readers, so leaving w
            # in PSUM would chain every dot product to its predecessor (a
            # ~1.4us legalized wait on the DVE sequencer per chunk).
            w_row = constp.tile([1, H], F32, name="w_row")
            nc.sync.dma_start(w_row[:], w_d.ap())
            ones_t = constp.tile([1, P], F32, name="ones_t")
            nc.gpsimd.memset(ones_t[:], 1.0)
            w_ps = psump.tile([P, H], F32, name="w_ps")  # spans 2 PSUM banks
            half = H // 2
            nc.tensor.matmul(w_ps[:, 0:half], ones_t[:], w_row[:, 0:half])
            nc.tensor.matmul(w_ps[:, half:H], ones_t[:], w_row[:, half:H])
            w_t = constp.tile([P, H], F32, name="w_t")
            # also serves as the probe: DVE observes the PE broadcast here, so
            # the dots carry only their own DMA wait (TRN2 TPB instruction
            # structs only encode a single sync wait).
            nc.vector.tensor_copy(w_t[:], w_ps[:])

            # identity for the PE transposes, built on-device (gpsimd is idle
            # and this keeps 64KiB off the serial DMA stream):
            # ones everywhere, then keep only where p - f == 0.
            ones_id = constp.tile([P, P], F32, name="ones_id")
            nc.gpsimd.memset(ones_id[:], 1.0)
            id_t = constp.tile([P, P], F32, name="id_t")
            nc.gpsimd.affine_select(
                out=id_t[:],
                in_=ones_id[:],
                pattern=[[-1, P]],
                compare_op=mybir.AluOpType.is_equal,
                fill=0.0,
                channel_multiplier=1,
            )
            id_probe = psump.tile([1, 1], F32, name="id_probe")
            nc.tensor.matmul(id_probe[:], id_t[:, 0:1], id_t[:, 0:1])

            # scores[p, ci, 0] = enc[b, 128*j + p, :] . w_enc for chunk
            # ci = j*BPC + b. Each accumulator slot is padded to 32B (SCORES_PAD
            # f32): adjacent slots would otherwise share a dependency-tracking
            # granule, chaining every dot to its predecessor (WAW) and forcing
            # a ~1.4us legalized wait onto the DVE sequencer per chunk.
            scores = smallp.tile([P, JT * BPC, SCORES_PAD], F32, name="scores")

            # One 512KiB DMA + one fused dot per (b, j) chunk: finest natural
            # granularity, so compute trails the DMA stream by only one chunk.
            # The trailing TAIL_CHUNKS chunks are split in half along H so the
            # final (un-overlapped) DVE ops shrink: the second-to-last dot is
            # what actually gates the last one.
            # j-major chunk order: all 8 batches of column-group j arrive
            # consecutively, so transpose j + exp j overlap the remaining
            # dot-product stream for j < JT-1.
            chunks = [(j, b) for j in range(JT) for b in range(BPC)]
            nt = len(chunks)
            if TAIL_CHUNKS > 0:
                partial = smallp.tile(
                    [P, TAIL_CHUNKS, LAST_SPLIT], F32, name="partial"
                )
            for ci, (j, b) in enumerate(chunks):
                split = ci >= nt - TAIL_CHUNKS and not dma_only and not compute_only
                if not split:
                    slices = [(0, H)]
                elif ci == nt - 1:
                    # asymmetric: the second piece (the only fully exposed
                    # dot in the whole kernel) is as small as possible
                    slices = [(0, H - LAST_PIECE), (H - LAST_PIECE, LAST_PIECE)]
                else:
                    slices = [(0, H // 2), (H // 2, H // 2)]
                nsplit = len(slices)
                for h in range(nsplit):
                    h0, hs = slices[h]
                    if ci == 0 and nsplit == 1:
                        et = et0  # DMA already issued before the w block
                    else:
                        et = ebufp.tile([P, hs], F32, name="et", tag="et")
                        # enc[b, 128j:128(j+1), hslice] rows are contiguous
                        if compute_only:
                            nc.sync.dma_start(et[0:1, 0:1], enc[b, 0:1, 0:1])
                        else:
                            nc.sync.dma_start(
                                et[:],
                                enc[b, j * P : (j + 1) * P, h0 : h0 + hs],
                            )
                    if dma_only:
                        continue
                    # fused elementwise-mult + free-dim reduction on DVE:
                    # et *= w ; accum = sum(...)
                    # The product is written in place over the enc tile
                    # (dead after this op): no scratch tile / WAW waits.
                    acc = (
                        scores[:, ci, 0:1]
                        if nsplit == 1
                        else partial[:, ci - (nt - TAIL_CHUNKS), h : h + 1]
                    )
                    nc.vector.scalar_tensor_tensor(
                        out=et[:],
                        in0=et[:],
                        scalar=1.0,
                        in1=w_t[:, h0 : h0 + hs],
                        op0=mybir.AluOpType.mult,
                        op1=mybir.AluOpType.mult,
                        accum_out=acc,
                    )
                if split:
                    # combine the partial sums of the split chunk
                    nc.vector.tensor_reduce(
                        out=scores[:, ci, 0:1],
                        in_=partial[:, ci - (nt - TAIL_CHUNKS), 0:nsplit],
                        axis=mybir.AxisListType.X,
                        op=mybir.AluOpType.add,
                    )

            if dma_only:
                # timing variant: just ship something to the output
                prob0 = smallp.tile([BPC, S], F32, name="prob0")
                nc.vector.tensor_copy(prob0[:], et[0:BPC, 0:S])
                nc.sync.dma_start(out_d.ap(), prob0[:])
            else:
                # transpose scores -> batch-on-partitions: 4 strided PE
                # transposes, each into its OWN PSUM bank (PSUM deps are
                # bank-granular: sharing one bank would serialize every
                # exp behind the last transpose).
                # psumT[j][b, p] = scores[p, b, j] = score(b, s=128j+p)
                psumT = [
                    psump.tile([BPC, P], F32, name=f"psumT{j}", tag=f"psumT{j}")
                    for j in range(JT)
                ]
                for j in range(JT):
                    nc.tensor.transpose(
                        psumT[j][:], scores[:, j * BPC : (j + 1) * BPC, 0], id_t[:]
                    )

                # softmax over the free dim (fully local per batch row).
                # No max-subtraction: scores for this problem are bounded well
                # inside fp32 exp range (|score| < ~60), and softmax(x) is
                # mathematically identical with or without the shift.
                # exp + partial row-sums per 128-column group, so the first
                # three groups overlap the still-running dot-product stream
                # (transpose j is ready as soon as batch 7's chunk j is done).
                expt = smallp.tile([BPC, S], F32, name="expt")
                sums4 = smallp.tile([BPC, JT], F32, name="sums4")
                for j in range(JT):
                    nc.scalar.activation(
                        out=expt[:, j * P : (j + 1) * P],
                        in_=psumT[j][:],
                        func=mybir.ActivationFunctionType.Exp,
                        bias=0.0,
                        scale=1.0,
                        accum_out=sums4[:, j : j + 1],
                    )
                sums = smallp.tile([BPC, 1], F32, name="sums")
                nc.vector.tensor_reduce(
                    out=sums[:],
                    in_=sums4[:],
                    axis=mybir.AxisListType.X,
                    op=mybir.AluOpType.add,
                )
                binv = smallp.tile([BPC, 1], F32, name="binv")
                nc.vector.reciprocal(binv[:], sums[:])
                prob = smallp.tile([BPC, S], F32, name="prob")
                nc.vector.tensor_scalar_mul(prob[:], expt[:], binv[:])

                nc.sync.dma_start(out_d.ap(), prob[:])

    nc.finalize()  # Bacc: runs compile() (wait legalization, reg alloc, ...)
    return nc


def kernel(hidden, encoder_outputs, W, b, v):
    global _compiled_nc, LAST_RESULTS

    # Fold the linear layer on host (fp64 for accuracy): only the
    # encoder-input slice of W survives the softmax. Force numpy so the fold
    # never runs through a jax device backend.
    W = np.asarray(W)
    v = np.asarray(v)
    w_enc = (v.astype(np.float64) @ W[:, 2 * H :].astype(np.float64)).astype(
        np.float32
    )
    w_row = np.ascontiguousarray(w_enc[None, :])
    enc = np.ascontiguousarray(np.asarray(encoder_outputs, dtype=np.float32))

    if _compiled_nc is None:
        _compiled_nc = _build_nc()

    in_maps = [
        {
            "enc_in": enc[c * BPC : (c + 1) * BPC],
            "w_in": w_row,
        }
        for c in range(N_CORES)
    ]
    LAST_RESULTS = run_bass_kernel_spmd(
        _compiled_nc, in_maps, core_ids=list(range(N_CORES))
    )
    out = np.concatenate([r["probs_out"] for r in LAST_RESULTS.results], axis=0)
    return out.astype(np.float32)



# revision 16
# speedup vs baseline: 1.7214x; 1.7214x over previous
"""Trainium2 Bass kernel for nn_Attn_58669253263845 (sparse_attention).

Reference computation:
    hidden2 = concat(hidden[0], hidden[1])                 # [B, 2H]
    attn_input = concat(bcast(hidden2), encoder_outputs)   # [B, S, 3H]
    energy = attn_input @ W.T + b                          # [B, S, H]
    scores = energy @ v                                    # [B, S]
    out = softmax(scores, axis=S)

Everything before the softmax is linear, so
    scores[b,s] = attn_input[b,s,:] . (v @ W) + v.b
                = hidden2[b,:] . w_hid + enc[b,s,:] . w_enc + v.b
The hidden/bias terms are constant per batch row and cancel in the softmax
over S.  Hence:
    out = softmax_s(enc[b,s,:] . w_enc),  w_enc = v @ W[:, 2H:3H]

The weight fold (1024x1024 matvec, weights only) is done on host in fp64;
the heavy part (64*512 dot products of length 1024 + softmax) runs on 8
NeuronCores, data-parallel over batch (8 batches per core).

Kernel shape (per core): the kernel is DMA-bound -- it must stream
8 batches x 512 x 1024 encoder values through SBUF once.  Two levers:

 1. fp16 on the wire.  enc and w_enc are rounded to fp16 on host,
    halving HBM->SBUF traffic.  Scores have std ~10 and fp16 rounding
    perturbs them by ~3e-3, an order of magnitude inside the 2e-2
    correctness gate (fp16 x fp16 products accumulate exactly in the
    PE's fp32 accumulators).
 2. dots on the PE, h on partitions.  The host uploads enc pre-permuted
    to [(j,b), p, (hb, s)] chunks (h = 128*hb + p, s_global = 128*j + s),
    so every chunk is one contiguous 256 KiB DMA and the chunk's dot
    products become 8 PSUM-accumulated stationary loads:
        scores[s, (j,b)] += chunk[:, hb]^T_{128x128} @ w[hb]_{128x1}
    The moving side is a single w column, so the PE trails the DMA
    stream with almost no engine time, and the DVE (whose fused dot
    gets no 16-bit speedup) drops out of the streaming path entirely.

Chunks stream j-major (all 8 batches of s-group j consecutively), so the
per-group epilogue (PSUM->SBUF score copy, PE transpose to batch-major,
exp+accum on ACT) overlaps the remaining stream for j < 3; only s-group
3's epilogue plus the final reciprocal+scale sit behind the last chunk.
"""

import sys
import types

import numpy as np
import concourse.bacc as bacc
import concourse.bass as bass
import concourse.mybir as mybir
import concourse.tile as tile
from concourse.bass_utils import run_bass_kernel_spmd

# run_bass_kernel_spmd(trace=True) (e.g. via BASS_TRACE=1 in the env)
# imports antenv.axon_hooks, which does not exist in this container. Register
# a stub returning "no hook" so tracing degrades gracefully instead of
# raising ModuleNotFoundError.
try:
    import antenv.axon_hooks  # noqa: F401
except ImportError:
    try:
        import antenv

        _stub = types.ModuleType("antenv.axon_hooks")
        _stub.get_axon_ntff_profile_hook = lambda: None  # type: ignore[attr-defined]
        sys.modules["antenv.axon_hooks"] = _stub
        antenv.axon_hooks = _stub
    except ImportError:
        pass

N_CORES = 8
B, S, H = 64, 512, 1024
P = 128             # SBUF partitions
BPC = B // N_CORES  # batches per core = 8
HB = H // P         # h-blocks per dot = 8
JT = S // P         # s-groups per batch = 4

F32 = mybir.dt.float32
F16 = mybir.dt.float16

_compiled_nc = None
LAST_RESULTS = None  # BassKernelResults of the most recent run (for profiling)




def _build_nc():
    """Per-core kernel: probs[BPC, S] = softmax_s(enc[BPC, S, H] @ w_enc).

    enc arrives pre-permuted as [(j,b), P, HB*P] fp16 (chunk (j,b) holds
    s-group j of batch b, h-within-block on partitions, (hb, s) on free),
    w_enc as [P, HB] fp16 (column hb = h-block hb's 128 weights).
    """
    # Bacc (not raw Bass): its compile() legalizes multi-wait instructions
    # into EventSemaphore waits (TRN2 allows only 1 sync wait per inst).
    nc = bacc.Bacc("TRN2", target_bir_lowering=False, debug=False)

    NCHUNK = JT * BPC
    enc_d = nc.dram_tensor("enc_in", [NCHUNK, P, HB * P], F16, kind="ExternalInput")
    w_d = nc.dram_tensor("w_in", [P, HB], F16, kind="ExternalInput")
    out_d = nc.dram_tensor("probs_out", [BPC, S], F32, kind="ExternalOutput")

    enc = enc_d.ap()

    with tile.TileContext(nc) as tc:
        with (
            tc.tile_pool(name="const", bufs=1) as constp,
            tc.tile_pool(name="ebuf", bufs=NCHUNK) as ebufp,
            tc.tile_pool(name="small", bufs=1) as smallp,
            tc.tile_pool(name="psum", bufs=1, space="PSUM") as psump,
        ):
            # The first enc chunk goes first in the DMA stream (its HWDGE
            # stage starts during the preamble and every later DMA's
            # descriptor-gen hides behind a running transfer); w's 2 KiB ride
            # second and land long before the first matmul needs them.
            # NOTE: one chunk per DMA -- a [2, P, F] -> [P, 2F] transfer maps
            # by FLAT element order (chunk 0 would land on partitions 0-63).
            t0 = ebufp.tile([P, HB * P], F16, name="e0_0", tag="e")
            nc.sync.dma_start(t0[:], enc[0])
            w_sb = constp.tile([P, HB], F16, name="w_sb")
            nc.sync.dma_start(w_sb[:], w_d.ap())

            # identity for the PE transposes, built on-device (gpsimd is idle
            # and this keeps 64KiB off the serial DMA stream).
            ones_id = constp.tile([P, P], F32, name="ones_id")
            nc.gpsimd.memset(ones_id[:], 1.0)
            id_t = constp.tile([P, P], F32, name="id_t")
            nc.gpsimd.affine_select(
                out=id_t[:],
                in_=ones_id[:],
                pattern=[[-1, P]],
                compare_op=mybir.AluOpType.is_equal,
                fill=0.0,
                channel_multiplier=1,
            )

            # scores_j[s, b] accumulate over the 8 h-blocks of each chunk.
            # One PSUM tile PER s-group: PSUM dependency tracking is
            # bank-granular, so a shared tile would chain every group's
            # epilogue behind the final group's matmuls.
            scores = [
                psump.tile([P, BPC], F32, name=f"scores{j}", tag=f"scores{j}")
                for j in range(JT)
            ]

            # DMA stream: one 256 KiB chunk per transfer, j-major (all 8
            # batches of an s-group consecutively) so each group's epilogue
            # overlaps the remaining stream.
            tiles = {(0, 0): t0[:]}
            for j in range(JT):
                for b in range(BPC):
                    if (j, b) in tiles:
                        continue
                    t = ebufp.tile([P, HB * P], F16, name=f"e{j}_{b}", tag="e")
                    nc.sync.dma_start(t[:], enc[j * BPC + b])
                    tiles[(j, b)] = t[:]

            # Dot products: 8 accumulated stationary loads per chunk. The
            # moving operand is one w column, so per-matmul engine time is a
            # single column pass.
            for j in range(JT):
                for b in range(BPC):
                    ch = tiles[(j, b)]
                    for hb in range(HB):
                        nc.tensor.matmul(
                            scores[j][:, b : b + 1],
                            ch[:, hb * P : (hb + 1) * P],
                            w_sb[:, hb : hb + 1],
                            start=(hb == 0),
                            stop=(hb == HB - 1),
                        )

            # Per-group epilogue: PSUM -> SBUF copy (PE transpose reads SBUF
            # only), transpose to batch-major, exp+partial row sum. Groups
            # j < 3 overlap the remaining DMA/matmul stream.
            # Each transpose lands in its OWN PSUM bank (PSUM deps are
            # bank-granular; sharing one would serialize the exps).
            scs = smallp.tile([P, NCHUNK], F32, name="scs")
            psumT = [
                psump.tile([BPC, P], F32, name=f"psumT{j}", tag=f"psumT{j}")
                for j in range(JT)
            ]
            expt = smallp.tile([BPC, S], F32, name="expt")
            sums4 = smallp.tile([BPC, JT], F32, name="sums4")
            for j in range(JT):
                cols = slice(j * BPC, (j + 1) * BPC)
                nc.vector.tensor_copy(scs[:, cols], scores[j][:])
                nc.tensor.transpose(psumT[j][:], scs[:, cols], id_t[:])
                # softmax without max-subtraction: |score| < ~60 is far
                # inside fp32 exp range and softmax is shift-invariant.
                nc.scalar.activation(
                    out=expt[:, j * P : (j + 1) * P],
                    in_=psumT[j][:],
                    func=mybir.ActivationFunctionType.Exp,
                    bias=0.0,
                    scale=1.0,
                    accum_out=sums4[:, j : j + 1],
                )

            sums = smallp.tile([BPC, 1], F32, name="sums")
            nc.vector.tensor_reduce(
                out=sums[:],
                in_=sums4[:],
                axis=mybir.AxisListType.X,
                op=mybir.AluOpType.add,
            )
            binv = smallp.tile([BPC, 1], F32, name="binv")
            nc.vector.reciprocal(binv[:], sums[:])
            prob = smallp.tile([BPC, S], F32, name="prob")
            nc.vector.tensor_scalar_mul(prob[:], expt[:], binv[:])

            nc.sync.dma_start(out_d.ap(), prob[:])

    nc.finalize()  # Bacc: runs compile() (wait legalization, reg alloc, ...)
    return nc


def kernel(hidden, encoder_outputs, W, b, v):
    global _compiled_nc, LAST_RESULTS

    # Fold the linear layer on host (fp64 for accuracy): only the
    # encoder-input slice of W survives the softmax. Force numpy so the fold
    # never runs through a jax device backend.
    W = np.asarray(W)
    v = np.asarray(v)
    w_enc = (v.astype(np.float64) @ W[:, 2 * H :].astype(np.float64)).astype(
        np.float32
    )
    # [P, HB] fp16: column hb holds weights for h = 128*hb .. 128*hb+127.
    w_t = np.ascontiguousarray(w_enc.astype(np.float16).reshape(HB, P).T)
    # Per-core chunk layout [(j,b), p, (hb, s)]: each (s-group, batch) chunk
    # is contiguous with h-within-block on partitions, so chunk DMAs are
    # plain contiguous transfers and the PE contracts over the partition dim.
    enc = np.asarray(encoder_outputs, dtype=np.float32).astype(np.float16)
    # [B, S, H] -> [B, JT, 128s, HB, 128p] -> [B, JT, 128p, HB, 128s]
    enc = enc.reshape(B, JT, P, HB, P).transpose(0, 1, 4, 3, 2)

    if _compiled_nc is None:
        _compiled_nc = _build_nc()

    in_maps = []
    for c in range(N_CORES):
        # [BPC, JT, p, hb, s] -> [(j, b), p, (hb, s)]
        core = enc[c * BPC : (c + 1) * BPC].transpose(1, 0, 2, 3, 4)
        in_maps.append(
            {
                "enc_in": np.ascontiguousarray(
                    core.reshape(JT * BPC, P, HB * P)
                ),
                "w_in": w_t,
            }
        )
    LAST_RESULTS = run_bass_kernel_spmd(
        _compiled_nc, in_maps, core_ids=list(range(N_CORES))
    )
    out = np.concatenate([r["probs_out"] for r in LAST_RESULTS.results], axis=0)
    return out.astype(np.float32)


# revision 34
# speedup vs baseline: 1.7499x; 1.0166x over previous
"""Trainium2 Bass kernel for nn_Attn_58669253263845 (sparse_attention).

Reference computation:
    hidden2 = concat(hidden[0], hidden[1])                 # [B, 2H]
    attn_input = concat(bcast(hidden2), encoder_outputs)   # [B, S, 3H]
    energy = attn_input @ W.T + b                          # [B, S, H]
    scores = energy @ v                                    # [B, S]
    out = softmax(scores, axis=S)

Everything before the softmax is linear, so
    scores[b,s] = attn_input[b,s,:] . (v @ W) + v.b
                = hidden2[b,:] . w_hid + enc[b,s,:] . w_enc + v.b
The hidden/bias terms are constant per batch row and cancel in the softmax
over S.  Hence:
    out = softmax_s(enc[b,s,:] . w_enc),  w_enc = v @ W[:, 2H:3H]

The weight fold (1024x1024 matvec, weights only) is done on host in fp64;
the heavy part (64*512 dot products of length 1024 + softmax) runs on 8
NeuronCores, data-parallel over batch (8 batches per core).

Kernel shape (per core): the kernel is DMA-bound -- it must stream
8 batches x 512 x 1024 encoder values through SBUF once.  Two levers:

 1. fp16 on the wire.  enc and w_enc are rounded to fp16 on host,
    halving HBM->SBUF traffic.  Scores have std ~10 and fp16 rounding
    perturbs them by ~3e-3, an order of magnitude inside the 2e-2
    correctness gate (fp16 x fp16 products accumulate exactly in the
    PE's fp32 accumulators).
 2. dots on the PE, h on partitions.  The host uploads enc pre-permuted
    to [(j,b), p, (hb, s)] chunks (h = 128*hb + p, s_global = 128*j + s),
    so every chunk is one contiguous 256 KiB DMA and the chunk's dot
    products become 8 PSUM-accumulated stationary loads:
        scores[s, (j,b)] += chunk[:, hb]^T_{128x128} @ w[hb]_{128x1}
    The moving side is a single w column, so the PE trails the DMA
    stream with almost no engine time, and the DVE (whose fused dot
    gets no 16-bit speedup) drops out of the streaming path entirely.

Chunks stream j-major (all 8 batches of s-group j consecutively), so the
per-group epilogue (PSUM->SBUF score copy, PE transpose to batch-major,
exp+accum on ACT) overlaps the remaining stream for j < 3; only s-group
3's epilogue plus the final reciprocal+scale sit behind the last chunk.
"""

import sys
import types

import numpy as np
import concourse.bacc as bacc
import concourse.bass as bass
import concourse.mybir as mybir
import concourse.tile as tile
from concourse.bass_utils import run_bass_kernel_spmd

# run_bass_kernel_spmd(trace=True) (e.g. via BASS_TRACE=1 in the env)
# imports antenv.axon_hooks, which does not exist in this container. Register
# a stub returning "no hook" so tracing degrades gracefully instead of
# raising ModuleNotFoundError.
try:
    import antenv.axon_hooks  # noqa: F401
except ImportError:
    try:
        import antenv

        _stub = types.ModuleType("antenv.axon_hooks")
        _stub.get_axon_ntff_profile_hook = lambda: None  # type: ignore[attr-defined]
        sys.modules["antenv.axon_hooks"] = _stub
        antenv.axon_hooks = _stub
    except ImportError:
        pass

N_CORES = 8
B, S, H = 64, 512, 1024
P = 128             # SBUF partitions
BPC = B // N_CORES  # batches per core = 8
HB = H // P         # h-blocks per dot = 8
JT = S // P         # s-groups per batch = 4

F32 = mybir.dt.float32
F16 = mybir.dt.float16

_compiled_nc = None
LAST_RESULTS = None  # BassKernelResults of the most recent run (for profiling)




def _build_nc():
    """Per-core kernel: probs[BPC, S] = softmax_s(enc[BPC, S, H] @ w_enc).

    enc arrives pre-permuted as [(j,b), P, HB*P] fp16 (chunk (j,b) holds
    s-group j of batch b, h-within-block on partitions, (hb, s) on free),
    w_enc as [P, HB] fp16 (column hb = h-block hb's 128 weights).
    """
    # Bacc (not raw Bass): its compile() legalizes multi-wait instructions
    # into EventSemaphore waits (TRN2 allows only 1 sync wait per inst).
    nc = bacc.Bacc("TRN2", target_bir_lowering=False, debug=False)

    NCHUNK = JT * BPC
    enc_d = nc.dram_tensor("enc_in", [NCHUNK, P, HB * P], F16, kind="ExternalInput")
    w_d = nc.dram_tensor("w_in", [P, HB], F16, kind="ExternalInput")
    out_d = nc.dram_tensor("probs_out", [BPC, S], F32, kind="ExternalOutput")

    enc = enc_d.ap()

    with tile.TileContext(nc) as tc:
        with (
            tc.tile_pool(name="const", bufs=1) as constp,
            tc.tile_pool(name="ebuf", bufs=NCHUNK) as ebufp,
            tc.tile_pool(name="small", bufs=1) as smallp,
            tc.tile_pool(name="psum", bufs=1, space="PSUM") as psump,
        ):
            w_sb = constp.tile([P, HB], F16, name="w_sb")

            # identity for the PE transposes, built on-device (gpsimd is idle
            # and this keeps 64KiB off the serial DMA stream).
            ones_id = constp.tile([P, P], F32, name="ones_id")
            nc.gpsimd.memset(ones_id[:], 1.0)
            id_t = constp.tile([P, P], F32, name="id_t")
            nc.gpsimd.affine_select(
                out=id_t[:],
                in_=ones_id[:],
                pattern=[[-1, P]],
                compare_op=mybir.AluOpType.is_equal,
                fill=0.0,
                channel_multiplier=1,
            )

            # scores_j[s, b] accumulate over the 8 h-blocks of each chunk.
            # One PSUM tile PER s-group: PSUM dependency tracking is
            # bank-granular, so a shared tile would chain every group's
            # epilogue behind the final group's matmuls.
            scores = [
                psump.tile([P, BPC], F32, name=f"scores{j}", tag=f"scores{j}")
                for j in range(JT)
            ]

            # DMA stream: one 256 KiB chunk per transfer, j-major (all 8
            # batches of an s-group consecutively) so each group's epilogue
            # overlaps the remaining stream.
            # NOTE: one chunk per DMA -- a [2, P, F] -> [P, 2F] transfer maps
            # by FLAT element order (chunk 0 would land on partitions 0-63).
            # w's 2 KiB DMA rides 9th: each 728 ns enc transfer banks 78 ns
            # of issue-pipeline margin (728 transfer vs 650 issue), and w's
            # 650 ns issue slot needs ~8 chunks of margin to hide; the
            # matmuls are 2 ns each and instantly catch up once w lands.
            tiles = {}
            for j in range(JT):
                for b in range(BPC):
                    t = ebufp.tile([P, HB * P], F16, name=f"e{j}_{b}", tag="e")
                    nc.sync.dma_start(t[:], enc[j * BPC + b])
                    tiles[(j, b)] = t[:]
                # w's 2 KiB DMA rides 9th: each 728 ns enc transfer banks
                # 78 ns of issue-pipeline margin (728 transfer vs 650 issue),
                # and w's 650 ns issue slot needs ~8 chunks of margin to
                # hide; the matmuls are 2 ns each and instantly catch up
                # once w lands.
                if j == 0:
                    nc.sync.dma_start(w_sb[:], w_d.ap())

            # Dot products: 8 accumulated stationary loads per chunk. The
            # moving operand is one w column, so per-matmul engine time is a
            # single column pass.
            for j in range(JT):
                for b in range(BPC):
                    ch = tiles[(j, b)]
                    for hb in range(HB):
                        nc.tensor.matmul(
                            scores[j][:, b : b + 1],
                            ch[:, hb * P : (hb + 1) * P],
                            w_sb[:, hb : hb + 1],
                            start=(hb == 0),
                            stop=(hb == HB - 1),
                        )

            # Per-group epilogue: PSUM -> SBUF copy (PE transpose reads SBUF
            # only), transpose to batch-major, exp+partial row sum. Groups
            # j < 3 overlap the remaining DMA/matmul stream.
            # Each transpose lands in its OWN PSUM bank (PSUM deps are
            # bank-granular; sharing one would serialize the exps).
            scs = smallp.tile([P, NCHUNK], F32, name="scs")
            psumT = [
                psump.tile([BPC, P], F32, name=f"psumT{j}", tag=f"psumT{j}")
                for j in range(JT)
            ]
            expt = smallp.tile([BPC, S], F32, name="expt")
            sums4 = smallp.tile([BPC, JT], F32, name="sums4")
            for j in range(JT):
                cols = slice(j * BPC, (j + 1) * BPC)
                nc.vector.tensor_copy(scs[:, cols], scores[j][:])
                nc.tensor.transpose(psumT[j][:], scs[:, cols], id_t[:])
                # softmax without max-subtraction: |score| < ~60 is far
                # inside fp32 exp range and softmax is shift-invariant.
                nc.scalar.activation(
                    out=expt[:, j * P : (j + 1) * P],
                    in_=psumT[j][:],
                    func=mybir.ActivationFunctionType.Exp,
                    bias=0.0,
                    scale=1.0,
                    accum_out=sums4[:, j : j + 1],
                )

            sums = smallp.tile([BPC, 1], F32, name="sums")
            nc.vector.tensor_reduce(
                out=sums[:],
                in_=sums4[:],
                axis=mybir.AxisListType.X,
                op=mybir.AluOpType.add,
            )
            binv = smallp.tile([BPC, 1], F32, name="binv")
            nc.vector.reciprocal(binv[:], sums[:])
            prob = smallp.tile([BPC, S], F32, name="prob")
            nc.vector.tensor_scalar_mul(prob[:], expt[:], binv[:])

            nc.sync.dma_start(out_d.ap(), prob[:])

    nc.finalize()  # Bacc: runs compile() (wait legalization, reg alloc, ...)
    return nc


def kernel(hidden, encoder_outputs, W, b, v):
    global _compiled_nc, LAST_RESULTS

    # Fold the linear layer on host (fp64 for accuracy): only the
    # encoder-input slice of W survives the softmax. Force numpy so the fold
    # never runs through a jax device backend.
    W = np.asarray(W)
    v = np.asarray(v)
    w_enc = (v.astype(np.float64) @ W[:, 2 * H :].astype(np.float64)).astype(
        np.float32
    )
    # [P, HB] fp16: column hb holds weights for h = 128*hb .. 128*hb+127.
    w_t = np.ascontiguousarray(w_enc.astype(np.float16).reshape(HB, P).T)
    # Per-core chunk layout [(j,b), p, (hb, s)]: each (s-group, batch) chunk
    # is contiguous with h-within-block on partitions, so chunk DMAs are
    # plain contiguous transfers and the PE contracts over the partition dim.
    enc = np.asarray(encoder_outputs, dtype=np.float32).astype(np.float16)
    # [B, S, H] -> [B, JT, 128s, HB, 128p] -> [B, JT, 128p, HB, 128s]
    enc = enc.reshape(B, JT, P, HB, P).transpose(0, 1, 4, 3, 2)

    if _compiled_nc is None:
        _compiled_nc = _build_nc()

    in_maps = []
    for c in range(N_CORES):
        # [BPC, JT, p, hb, s] -> [(j, b), p, (hb, s)]
        core = enc[c * BPC : (c + 1) * BPC].transpose(1, 0, 2, 3, 4)
        in_maps.append(
            {
                "enc_in": np.ascontiguousarray(
                    core.reshape(JT * BPC, P, HB * P)
                ),
                "w_in": w_t,
            }
        )
    LAST_RESULTS = run_bass_kernel_spmd(
        _compiled_nc, in_maps, core_ids=list(range(N_CORES))
    )
    out = np.concatenate([r["probs_out"] for r in LAST_RESULTS.results], axis=0)
    return out.astype(np.float32)
